# revision 1
# baseline (speedup 1.0000x reference)
"""TRN2 Bass kernel for nn_Blur: upfirdn2d(pad=(2,1)) with a separable 4x4
binomial FIR, x shape (8, 256, 256, 256) f32, depthwise per (n, c) plane.

Strategy
--------
Batch-parallel across the 8 NeuronCores (core i gets x[i]).

The FIR is separable: out = T_H^T @ X @ T_W per (c) plane, where T_H/T_W are
256x256 banded Toeplitz matrices (band k1[0..3] on diagonals -1..+2, zero
boundary = the reference's zero padding).

Both passes run on the TensorEngine with the *data* as the stationary
operand (lhsT) and the Toeplitz as the moving operand (rhs):

  pass1:  Y^T = X^T @ T_H      (lhsT = X tile   [h_in=128, w=128],
                                rhs  = T_H blk  [h_in=128, h'=256])
  pass2:  Z   = Y  @ T_W       (lhsT = Y^T tile [w_in=128, h'=128],
                                rhs  = T_W blk  [w_in=128, w'=256])

so no transposes are needed: pass1 naturally yields Y^T, pass2 naturally
yields Z in output layout.

Precision: pass1 data is split on the host into bf16 hi + lo halves
(x = hi + lo to ~2^-18) and the Toeplitz entries are exact in bf16, so
pass1 is fp32-accurate at bf16 matmul speed. For pass2, Y is re-split
on-device (ScalarE cast + VectorE subtract), keeping pass2 exact too.

DMA-efficiency tricks (descriptor size is what matters on TRN2):
 * inputs are pre-swizzled on the host into the exact SBUF tile layout
   [group][partition][hi/lo][c][hb][w] -> one 2 MiB DMA per group of
   CG channels with 16 KiB contiguous runs per partition.
 * T_H's columns are permuted (all even h' then all odd h'), so pass2's
   two output tiles hold even rows / odd rows on matching partitions;
   partition p then stores output rows (2p, 2p+1) of each channel as one
   2 KiB contiguous DRAM run.
"""
import numpy as np
import ml_dtypes

import concourse.bacc as bacc
import concourse.mybir as mybir
from concourse.tile import TileContext
from concourse.bass_utils import run_bass_kernel_spmd

N, C, H, W = 8, 256, 256, 256
P = 128          # partition size
NCORES = 8
# band: T[i, i+d] = k1[d+1], d in {-1, 0, 1, 2}
BAND_LO, BAND_HI = -1, 2
# pass2 (T_W, natural order): nonzero column ranges of the two 128-row blocks
BLK_COLS = [(0, P + BAND_HI), (P + BAND_LO, 2 * P)]   # [0,130), [127,256)
# pass1 (T_H, even/odd-permuted columns): nonzero column spans per 128-row
# block. block0 touches h' <= 129 -> evens [0,65) + odds [128,193);
# block1 touches h' >= 127 -> evens [64,128) + odds [191,256).
P1_COLS = [(0, 193), (64, 256)]

CG = 8           # channels per DMA group

_CACHE = {}


def _factor_kernel(k2: np.ndarray):
    """Rank-1 factorization k2 = kh (x) kw (float64)."""
    k2 = np.asarray(k2, dtype=np.float64)
    u, s, vt = np.linalg.svd(k2)
    kh = u[:, 0] * np.sqrt(s[0])
    kw = vt[0] * np.sqrt(s[0])
    if kh.sum() < 0:
        kh, kw = -kh, -kw
    return kh, kw


def _toeplitz(n: int, k1: np.ndarray) -> np.ndarray:
    """T[i, j] = k1[j - i + 1] for 0 <= j-i+1 < 4, zero elsewhere."""
    t = np.zeros((n, n), dtype=np.float64)
    for d in range(BAND_LO, BAND_HI + 1):
        i = np.arange(max(0, -d), min(n, n - d))
        t[i, i + d] = k1[d + 1]
    return t


def _build(n_ch: int, cg: int = CG, reps: int = 1):
    """Build + compile the per-core Bass program (SPMD, one core's slice).

    reps > 1 repeats the whole channel loop (idempotent) — a timing aid
    that amortizes dispatch overhead out of wall-clock measurements.
    """
    nc = bacc.Bacc("TRN2", target_bir_lowering=False)

    bf16 = mybir.dt.bfloat16
    f32 = mybir.dt.float32

    assert n_ch % cg == 0
    ng = n_ch // cg
    # [group][partition][hl][c][hb][w] pre-swizzled input, bf16 hi+lo
    xin = nc.declare_dram_parameter("xin", [ng, P, 2 * cg * 2 * W], bf16,
                                    isOutput=False)
    th = nc.declare_dram_parameter("th", [2, P, H], bf16, isOutput=False)
    tw = nc.declare_dram_parameter("tw", [2, P, W], bf16, isOutput=False)
    # h split as (p, s): h = 2p + s
    out = nc.declare_dram_parameter("out", [n_ch, P, 2, W], f32,
                                    isOutput=True)

    with TileContext(nc) as tc:
        with (tc.tile_pool(name="const", bufs=1) as cpool,
              tc.tile_pool(name="xin_p", bufs=3) as xpool,
              tc.tile_pool(name="mid", bufs=8) as mpool,
              tc.tile_pool(name="zout", bufs=3) as zpool,
              tc.tile_pool(name="psy", bufs=4, space="PSUM") as pypool,
              tc.tile_pool(name="psz", bufs=4, space="PSUM") as pzpool):

            tth = [cpool.tile([P, H], bf16, name=f"tth{b}", tag=f"tth{b}")
                   for b in range(2)]
            ttw = [cpool.tile([P, W], bf16, name=f"ttw{b}", tag=f"ttw{b}")
                   for b in range(2)]
            for b in range(2):
                nc.sync.dma_start(out=tth[b][:, :], in_=th[b])
                nc.sync.dma_start(out=ttw[b][:, :], in_=tw[b])

            for g in [gg for _ in range(reps) for gg in range(ng)]:
                # one contiguous 2 MiB load: [128, 16 KiB]
                tx = xpool.tile([P, 2 * cg * 2 * W], bf16, name="tx",
                                tag="tx")
                nc.sync.dma_start(out=tx[:, :], in_=xin[g])

                tz = zpool.tile([P, cg * 2 * W], f32, name="tz", tag="tz")

                for ci in range(cg):
                    # ---- pass1: Y^T[wb] = sum_hb,hl X[hl,hb,:,wb]^T @ TH[hb]
                    tyh = mpool.tile([P, 2 * H], bf16, name="tyh", tag="tyh")
                    tyl = mpool.tile([P, 2 * H], bf16, name="tyl", tag="tyl")
                    for wb in range(2):
                        py = pypool.tile([P, H], f32, name="py", tag="py")
                        first = True
                        P1I = [[(0, 65), (128, 193)],
                               [(64, 128), (191, 256)]]
                        for hb in range(2):
                            for hl in range(2):
                                off = (hl * cg + ci) * 2 * W + hb * W + wb * P
                                ivs = [(0, H)] if first else P1I[hb]
                                for ivi, (lo, hi) in enumerate(ivs):
                                    nc.tensor.matmul(
                                        py[:, lo:hi], tx[:, off:off + P],
                                        tth[hb][:, lo:hi],
                                        start=first,
                                        stop=(hb == 1 and hl == 1
                                              and ivi == len(ivs) - 1))
                                    first = False
                        # split Y into bf16 hi+lo (exact to ~2^-18)
                        ysl = slice(wb * H, (wb + 1) * H)
                        nc.scalar.copy(tyh[:, ysl], py[:, :])
                        nc.vector.tensor_sub(tyl[:, ysl], py[:, :],
                                             tyh[:, ysl])

                    # ---- pass2: Z[s] = sum_wb,(h/l) Y^T[wb,:,s]^T @ TW[wb]
                    # s = 0: even output rows (partition p = row 2p),
                    # s = 1: odd  output rows (partition p = row 2p+1).
                    for s in range(2):
                        pz = pzpool.tile([P, W], f32, name="pz", tag="pz")
                        first = True
                        for wb in range(2):
                            lo, hi = (0, W) if first else BLK_COLS[wb]
                            ysl = slice(wb * H + s * P, wb * H + s * P + P)
                            for ty in (tyh, tyl):
                                nc.tensor.matmul(
                                    pz[:, lo:hi], ty[:, ysl],
                                    ttw[wb][:, lo:hi],
                                    start=first,
                                    stop=(wb == 1 and ty is tyl))
                                first = False
                                lo, hi = BLK_COLS[wb]
                        zsl = slice(ci * 2 * W + s * W, ci * 2 * W + s * W + W)
                        if s == 0:
                            nc.vector.tensor_copy(tz[:, zsl], pz[:, :])
                        else:
                            nc.scalar.copy(tz[:, zsl], pz[:, :])

                # ---- store cg channels: partition p -> rows (2p, 2p+1)
                dst = out[g * cg:(g + 1) * cg].rearrange("c p s w -> p c s w")
                nc.sync.dma_start(
                    out=dst,
                    in_=tz[:, :].rearrange("p (c s w) -> p c s w", c=cg, s=2))
    nc.compile()
    return nc


def _get_nc(n_ch: int):
    key = (n_ch, CG)
    if key not in _CACHE:
        _CACHE[key] = _build(n_ch)
    return _CACHE[key]


def _perm_evenodd(n: int) -> np.ndarray:
    return np.concatenate([np.arange(0, n, 2), np.arange(1, n, 2)])


def _prep_inputs(x: np.ndarray, k2: np.ndarray, n_ch: int):
    cg = CG
    ng = n_ch // cg
    kh, kw = _factor_kernel(k2)
    th64 = _toeplitz(H, kh)[:, _perm_evenodd(H)]   # permuted columns
    tw64 = _toeplitz(W, kw)
    th = th64.astype(ml_dtypes.bfloat16).reshape(2, P, H)
    tw = tw64.astype(ml_dtypes.bfloat16).reshape(2, P, W)
    th = np.ascontiguousarray(th)
    tw = np.ascontiguousarray(tw)

    x32 = np.asarray(x, dtype=np.float32)
    xhi = x32.astype(ml_dtypes.bfloat16)
    xlo = (x32 - xhi.astype(np.float32)).astype(ml_dtypes.bfloat16)
    # [n, c, h, w] -> [n, g, c', hb, p, w] -> [n, g, p, (hl, c', hb, w)]
    xhi = xhi.reshape(N, ng, cg, 2, P, W)
    xlo = xlo.reshape(N, ng, cg, 2, P, W)
    xin = np.stack([xhi, xlo], axis=3)            # [n, g, c', hl, hb, p, w]
    xin = xin.transpose(0, 1, 5, 3, 2, 4, 6)      # [n, g, p, hl, c', hb, w]
    xin = np.ascontiguousarray(xin).reshape(N, ng, P, 2 * cg * 2 * W)

    in_maps = []
    for i in range(NCORES):
        in_maps.append({"xin": xin[i], "th": th, "tw": tw})
    return in_maps


def _run(x: np.ndarray, k2: np.ndarray, trace: bool = False):
    n_ch = C
    nc = _get_nc(n_ch)
    in_maps = _prep_inputs(x, k2, n_ch)
    r = run_bass_kernel_spmd(nc, in_maps, core_ids=list(range(NCORES)),
                             trace=trace)
    # out [n_ch, P, 2, W]: h = 2p + s -> natural reshape
    outs = [r.results[i]["out"].reshape(n_ch, H, W) for i in range(NCORES)]
    return np.stack(outs, axis=0), r


def kernel(x: np.ndarray, kernel: np.ndarray) -> np.ndarray:
    out, _ = _run(x, kernel, trace=False)
    return out



# revision 2
# speedup vs baseline: 1.3134x; 1.3134x over previous
"""TRN2 Bass kernel for nn_Blur: upfirdn2d(pad=(2,1)) with a separable 4x4
binomial FIR, x shape (8, 256, 256, 256) f32, depthwise per (n, c) plane.

Strategy
--------
Batch-parallel across the 8 NeuronCores (core i gets x[i]).

The FIR is separable: out = T_H^T @ X @ T_W per channel plane, where
T_H/T_W are 256x256 banded Toeplitz matrices (band k1[0..3] on diagonals
-1..+2, zero boundary = the reference's zero padding).

Both passes run on the TensorEngine with the *data* as the stationary
operand (lhsT) and the Toeplitz as the moving operand (rhs):

  pass1:  Y^T = X^T @ T_H      (lhsT = X tile   [h_in=128, w=128],
                                rhs  = T_H blk  [h_in=128, h'=256])
  pass2:  Z   = Y  @ T_W       (lhsT = Y^T tile [w_in=128, h'=128],
                                rhs  = T_W blk  [w_in=128, w'=256])

so no transposes are needed: pass1 naturally yields Y^T, pass2 naturally
yields Z in output layout.

Precision: the rel-err budget is 2e-2; plain bf16 data (~1e-3 end to end)
is comfortably inside it, so x is cast to bf16 on the host and every
device tensor except PSUM is bf16 — half the HBM traffic and half the PE
work of an fp32-exact split scheme.

DMA efficiency: both input and output DRAM tensors are laid out so each
SBUF partition's slice is one contiguous 16 KiB DRAM run per group of
CG=16 channels (input pre-swizzled on the host, output gathered on the
host), keeping descriptor counts minimal.  The banded structure makes
each Toeplitz 128-block touch a single contiguous column interval
([0,130) / [127,256)), so each accumulation needs just 2 matmuls: one
full-width N=256 (start=True zero-fills the tail) + one N=129.
"""
import numpy as np
import ml_dtypes

import concourse.bacc as bacc
import concourse.mybir as mybir
from concourse.tile import TileContext
from concourse.bass_utils import run_bass_kernel_spmd

N, C, H, W = 8, 256, 256, 256
P = 128          # partition size
NCORES = 8
# band: T[i, i+d] = k1[d+1], d in {-1, 0, 1, 2}
BAND_LO, BAND_HI = -1, 2
# nonzero column ranges of the two 128-row Toeplitz blocks (natural order)
BLK_COLS = [(0, P + BAND_HI), (P + BAND_LO, 2 * P)]   # [0,130), [127,256)

CG = 16          # channels per DMA group

_CACHE = {}


def _factor_kernel(k2: np.ndarray):
    """Rank-1 factorization k2 = kh (x) kw (float64)."""
    k2 = np.asarray(k2, dtype=np.float64)
    u, s, vt = np.linalg.svd(k2)
    kh = u[:, 0] * np.sqrt(s[0])
    kw = vt[0] * np.sqrt(s[0])
    if kh.sum() < 0:
        kh, kw = -kh, -kw
    return kh, kw


def _toeplitz(n: int, k1: np.ndarray) -> np.ndarray:
    """T[i, j] = k1[j - i + 1] for 0 <= j-i+1 < 4, zero elsewhere."""
    t = np.zeros((n, n), dtype=np.float64)
    for d in range(BAND_LO, BAND_HI + 1):
        i = np.arange(max(0, -d), min(n, n - d))
        t[i, i + d] = k1[d + 1]
    return t


def _build(n_ch: int, cg: int = CG, reps: int = 1):
    """Build + compile the per-core Bass program (SPMD, one core's slice).

    reps > 1 repeats the whole channel loop (idempotent) — a timing aid
    that amortizes dispatch overhead out of wall-clock measurements.
    """
    nc = bacc.Bacc("TRN2", target_bir_lowering=False)

    bf16 = mybir.dt.bfloat16

    assert n_ch % cg == 0
    ng = n_ch // cg
    # [group][partition][c][hb][w] pre-swizzled bf16 input
    xin = nc.declare_dram_parameter("xin", [ng, P, cg * 2 * W], bf16,
                                    isOutput=False)
    th = nc.declare_dram_parameter("th", [2, P, H], bf16, isOutput=False)
    tw = nc.declare_dram_parameter("tw", [2, P, W], bf16, isOutput=False)
    # [group][partition][c][s][w] partition-major output: h = s*128 + p
    out = nc.declare_dram_parameter("out", [ng, P, cg * 2 * W], bf16,
                                    isOutput=True)

    with TileContext(nc) as tc:
        with (tc.tile_pool(name="const", bufs=1) as cpool,
              tc.tile_pool(name="xin_p", bufs=3) as xpool,
              tc.tile_pool(name="mid", bufs=6) as mpool,
              tc.tile_pool(name="zout", bufs=3) as zpool,
              tc.tile_pool(name="psy", bufs=3, space="PSUM") as pypool,
              tc.tile_pool(name="psz", bufs=3, space="PSUM") as pzpool):

            tth = [cpool.tile([P, H], bf16, name=f"tth{b}", tag=f"tth{b}")
                   for b in range(2)]
            ttw = [cpool.tile([P, W], bf16, name=f"ttw{b}", tag=f"ttw{b}")
                   for b in range(2)]
            for b in range(2):
                nc.sync.dma_start(out=tth[b][:, :], in_=th[b])
                nc.sync.dma_start(out=ttw[b][:, :], in_=tw[b])

            f32 = mybir.dt.float32
            for g in [gg for _ in range(reps) for gg in range(ng)]:
                # one contiguous load: [128 x 16 KiB]
                tx = xpool.tile([P, cg * 2 * W], bf16, name="tx", tag="tx")
                nc.sync.dma_start(out=tx[:, :], in_=xin[g])

                tz = zpool.tile([P, cg * 2 * W], bf16, name="tz", tag="tz")

                for ci in range(cg):
                    # ---- pass1: Y^T[wb] = sum_hb X[hb,:,wb]^T @ TH[hb]
                    # py[:, wb*256 + h'] = Y[h', wb*128 + p]
                    py = pypool.tile([P, 2 * H], f32, name="py", tag="py")
                    for wb in range(2):
                        for hb in range(2):
                            lo, hi = (0, H) if hb == 0 else BLK_COLS[1]
                            off = (ci * 2 + hb) * W + wb * P
                            nc.tensor.matmul(
                                py[:, wb * H + lo:wb * H + hi],
                                tx[:, off:off + P],
                                tth[hb][:, lo:hi],
                                start=(hb == 0), stop=(hb == 1))
                    ty = mpool.tile([P, 2 * H], bf16, name="ty", tag="ty")
                    nc.vector.tensor_copy(ty[:, :], py[:, :])

                    # ---- pass2: Z[s*128+p, w'] = sum_wb Y^T[wb,:,s]^T @ TW[wb]
                    # pz[:, s*256 + w'] = Z[s*128 + p, w']
                    pz = pzpool.tile([P, 2 * W], f32, name="pz", tag="pz")
                    for s in range(2):
                        for wb in range(2):
                            lo, hi = (0, W) if wb == 0 else BLK_COLS[1]
                            nc.tensor.matmul(
                                pz[:, s * W + lo:s * W + hi],
                                ty[:, wb * H + s * P:wb * H + s * P + P],
                                ttw[wb][:, lo:hi],
                                start=(wb == 0), stop=(wb == 1))
                    nc.scalar.copy(tz[:, ci * 2 * W:(ci + 1) * 2 * W],
                                   pz[:, :])

                # ---- store: one contiguous [128 x 16 KiB] run
                nc.sync.dma_start(out=out[g], in_=tz[:, :])
    nc.compile()
    return nc


def _get_nc(n_ch: int):
    key = (n_ch, CG)
    if key not in _CACHE:
        _CACHE[key] = _build(n_ch)
    return _CACHE[key]


def _prep_inputs(x: np.ndarray, k2: np.ndarray, n_ch: int):
    cg = CG
    ng = n_ch // cg
    kh, kw = _factor_kernel(k2)
    th = _toeplitz(H, kh).astype(ml_dtypes.bfloat16).reshape(2, P, H)
    tw = _toeplitz(W, kw).astype(ml_dtypes.bfloat16).reshape(2, P, W)
    th = np.ascontiguousarray(th)
    tw = np.ascontiguousarray(tw)

    xb = np.asarray(x).astype(ml_dtypes.bfloat16)
    # [n, (g c), (hb p), w] -> [n, g, p, c, hb, w]
    xb = xb.reshape(N, ng, cg, 2, P, W).transpose(0, 1, 4, 2, 3, 5)
    xin = np.ascontiguousarray(xb).reshape(N, ng, P, cg * 2 * W)

    in_maps = []
    for i in range(NCORES):
        in_maps.append({"xin": xin[i], "th": th, "tw": tw})
    return in_maps


def _unpack_out(raw: np.ndarray, n_ch: int) -> np.ndarray:
    """[ng, P, cg*2*W] bf16 -> [n_ch, H, W] f32; h = s*128 + p."""
    cg = CG
    ng = n_ch // cg
    r = raw.reshape(ng, P, cg, 2, W).transpose(0, 2, 3, 1, 4)
    return np.ascontiguousarray(r).reshape(n_ch, H, W).astype(np.float32)


def _run(x: np.ndarray, k2: np.ndarray, trace: bool = False):
    n_ch = C
    nc = _get_nc(n_ch)
    in_maps = _prep_inputs(x, k2, n_ch)
    r = run_bass_kernel_spmd(nc, in_maps, core_ids=list(range(NCORES)),
                             trace=trace)
    outs = [_unpack_out(r.results[i]["out"], n_ch) for i in range(NCORES)]
    return np.stack(outs, axis=0), r


def kernel(x: np.ndarray, kernel: np.ndarray) -> np.ndarray:
    out, _ = _run(x, kernel, trace=False)
    return out


# revision 23
# speedup vs baseline: 3.9704x; 3.0231x over previous
"""TRN2 Bass kernel for nn_Blur: upfirdn2d(pad=(2,1)) with a separable 4x4
binomial FIR, x shape (8, 256, 256, 256) f32, depthwise per (n, c) plane.

Strategy
--------
Batch-parallel across the 8 NeuronCores (core i gets x[i]).

The FIR is separable: out = T_H^T @ X @ T_W per channel plane, where
T_H/T_W are 256x256 banded Toeplitz matrices (band k1[0..3] on diagonals
-1..+2, zero boundary = the reference's zero padding).

Both passes run on the TensorEngine with the *data* as the stationary
operand (lhsT) and the Toeplitz as the moving operand (rhs):

  pass1:  Y^T = X^T @ T_H      (lhsT = X tile   [h_in=128, w=128],
                                rhs  = T_H blk  [h_in=128, h'=256])
  pass2:  Z   = Y  @ T_W       (lhsT = Y^T tile [w_in=128, h'=128],
                                rhs  = T_W blk  [w_in=128, w'=256])

so no transposes are needed: pass1 naturally yields Y^T, pass2 naturally
yields Z in output layout.

Precision: the rel-err budget is 2e-2; plain bf16 data (~1e-3 end to end)
is comfortably inside it, so x is cast to bf16 on the host and every
device tensor except PSUM is bf16 — half the HBM traffic and half the PE
work of an fp32-exact split scheme.

DMA efficiency: both input and output DRAM tensors are laid out so each
SBUF partition's slice is one contiguous 16 KiB DRAM run per group of
CG=16 channels (input pre-swizzled on the host, output gathered on the
host), keeping descriptor counts minimal.  The banded structure makes
each Toeplitz 128-block touch a single contiguous column interval
([0,130) / [127,256)), so each accumulation needs just 2 matmuls: one
full-width N=256 (start=True zero-fills the tail) + one N=129.
"""
import numpy as np
import ml_dtypes

import concourse.bacc as bacc
import concourse.mybir as mybir
from concourse.tile import TileContext
from concourse.bass_utils import run_bass_kernel_spmd

N, C, H, W = 8, 256, 256, 256
P = 128          # partition size
NCORES = 8
# band: T[i, i+d] = k1[d+1], d in {-1, 0, 1, 2}
BAND_LO, BAND_HI = -1, 2
# nonzero column ranges of the two 128-row Toeplitz blocks (natural order)
BLK_COLS = [(0, P + BAND_HI), (P + BAND_LO, 2 * P)]   # [0,130), [127,256)

CG = 16          # channels per DMA group

_CACHE = {}


def _factor_kernel(k2: np.ndarray):
    """Rank-1 factorization k2 = kh (x) kw (float64)."""
    k2 = np.asarray(k2, dtype=np.float64)
    u, s, vt = np.linalg.svd(k2)
    kh = u[:, 0] * np.sqrt(s[0])
    kw = vt[0] * np.sqrt(s[0])
    if kh.sum() < 0:
        kh, kw = -kh, -kw
    return kh, kw


def _toeplitz(n: int, k1: np.ndarray) -> np.ndarray:
    """T[i, j] = k1[j - i + 1] for 0 <= j-i+1 < 4, zero elsewhere."""
    t = np.zeros((n, n), dtype=np.float64)
    for d in range(BAND_LO, BAND_HI + 1):
        i = np.arange(max(0, -d), min(n, n - d))
        t[i, i + d] = k1[d + 1]
    return t


def _build(n_ch: int, cg: int = CG, reps: int = 1, *,
           dma_split: bool = True, skip_compute: bool = False,
           skip_dma: bool = False, bufs: tuple = (3, 3),
           only: str | None = None, skew: int = 2,
           p2alt: bool = False, swap_q: bool = False):
    """Build + compile the per-core Bass program (SPMD, one core's slice).

    reps > 1 repeats the whole channel loop (idempotent) — a timing aid
    that amortizes dispatch overhead out of wall-clock measurements.
    dma_split: issue stores on the Activation HWDGE queue (loads stay on
    SP) so the two big streams ride different hardware queues.
    skip_compute / skip_dma: ablation variants for bottleneck attribution.
    """
    nc = bacc.Bacc("TRN2", target_bir_lowering=False)

    bf16 = mybir.dt.bfloat16

    assert n_ch % cg == 0
    ng = n_ch // cg
    # [group][partition][c][hb][w] pre-swizzled bf16 input
    xin = nc.declare_dram_parameter("xin", [ng, P, cg * 2 * W], bf16,
                                    isOutput=False)
    th = nc.declare_dram_parameter("th", [2, P, H], bf16, isOutput=False)
    tw = nc.declare_dram_parameter("tw", [2, P, W], bf16, isOutput=False)
    # [group][partition][c][s][w] partition-major output: h = s*128 + p
    out = nc.declare_dram_parameter("out", [ng, P, cg * 2 * W], bf16,
                                    isOutput=True)

    with TileContext(nc) as tc:
        with (tc.tile_pool(name="const", bufs=1) as cpool,
              tc.tile_pool(name="xin_p", bufs=bufs[0]) as xpool,
              tc.tile_pool(name="mid", bufs=6) as mpool,
              tc.tile_pool(name="zout", bufs=bufs[1]) as zpool,
              tc.tile_pool(name="psy", bufs=3, space="PSUM") as pypool,
              tc.tile_pool(name="psz", bufs=3, space="PSUM") as pzpool):

            tth = [cpool.tile([P, H], bf16, name=f"tth{b}", tag=f"tth{b}")
                   for b in range(2)]
            ttw = [cpool.tile([P, W], bf16, name=f"ttw{b}", tag=f"ttw{b}")
                   for b in range(2)]
            for b in range(2):
                nc.sync.dma_start(out=tth[b][:, :], in_=th[b])
                nc.sync.dma_start(out=ttw[b][:, :], in_=tw[b])

            f32 = mybir.dt.float32

            def emit_group(g):
                # one contiguous load: [128 x 16 KiB]
                tx = xpool.tile([P, cg * 2 * W], bf16, name="tx", tag="tx")
                if not skip_dma and only != "store":
                    (nc.scalar if swap_q else nc.sync).dma_start(
                        out=tx[:, :], in_=xin[g])

                tz = zpool.tile([P, cg * 2 * W], bf16, name="tz", tag="tz")
                if skip_compute and only == "store":
                    nc.vector.memset(tz[:, :], 0.0)

                def pass1(ci):
                    # ---- pass1: Y^T[wb] = sum_hb X[hb,:,wb]^T @ TH[hb]
                    # py[:, wb*256 + h'] = Y[h', wb*128 + p]
                    py = pypool.tile([P, 2 * H], f32, name="py", tag="py")
                    for wb in range(2):
                        for hb in range(2):
                            lo, hi = (0, H) if hb == 0 else BLK_COLS[1]
                            off = (ci * 2 + hb) * W + wb * P
                            nc.tensor.matmul(
                                py[:, wb * H + lo:wb * H + hi],
                                tx[:, off:off + P],
                                tth[hb][:, lo:hi],
                                start=(hb == 0), stop=(hb == 1))
                    ty = mpool.tile([P, 2 * H], bf16, name="ty", tag="ty")
                    nc.vector.tensor_copy(ty[:, :], py[:, :])
                    return ty

                def pass2(ci, ty):
                    # ---- pass2: Z[s*128+p, w'] = sum_wb Y^T[wb,:,s]^T @ TW[wb]
                    # pz[:, s*256 + w'] = Z[s*128 + p, w']
                    pz = pzpool.tile([P, 2 * W], f32, name="pz", tag="pz")
                    for s in range(2):
                        for wb in range(2):
                            lo, hi = (0, W) if wb == 0 else BLK_COLS[1]
                            nc.tensor.matmul(
                                pz[:, s * W + lo:s * W + hi],
                                ty[:, wb * H + s * P:wb * H + s * P + P],
                                ttw[wb][:, lo:hi],
                                start=(wb == 0), stop=(wb == 1))
                    dst = tz[:, ci * 2 * W:(ci + 1) * 2 * W]
                    if p2alt and ci % 2:
                        nc.vector.tensor_copy(dst, pz[:, :])
                    else:
                        nc.scalar.copy(dst, pz[:, :])

                # software-pipeline: pass1 of channel ci+skew runs ahead of
                # pass2 of channel ci so the PE never waits on the DVE
                # PSUM->SBUF copy between passes (PE queue is in-order).
                pend = []
                for ci in range(cg) if not skip_compute else []:
                    pend.append((ci, pass1(ci)))
                    if len(pend) > skew:
                        pass2(*pend.pop(0))
                for item in pend:
                    pass2(*item)

                # ---- store: one contiguous [128 x 16 KiB] run
                if not skip_dma and only != "load":
                    src = tx if (skip_compute and only != "store") else tz
                    if not dma_split:
                        eng = nc.sync
                    else:
                        eng = nc.sync if swap_q else nc.scalar
                    eng.dma_start(out=out[g], in_=src[:, :])

            if reps > 1:
                # hardware loop: repeat the (idempotent) channel loop
                # in-NEFF for wall-clock timing without code growth
                with tc.For_i(0, reps, 1):
                    for g in range(ng):
                        emit_group(g)
            else:
                for g in range(ng):
                    emit_group(g)
    nc.compile()
    return nc


def _get_nc(n_ch: int):
    key = (n_ch, CG)
    if key not in _CACHE:
        _CACHE[key] = _build(n_ch)
    return _CACHE[key]


def _prep_inputs(x: np.ndarray, k2: np.ndarray, n_ch: int, cg: int = CG):
    ng = n_ch // cg
    kh, kw = _factor_kernel(k2)
    th = _toeplitz(H, kh).astype(ml_dtypes.bfloat16).reshape(2, P, H)
    tw = _toeplitz(W, kw).astype(ml_dtypes.bfloat16).reshape(2, P, W)
    th = np.ascontiguousarray(th)
    tw = np.ascontiguousarray(tw)

    xb = np.asarray(x).astype(ml_dtypes.bfloat16)
    # [n, (g c), (hb p), w] -> [n, g, p, c, hb, w]
    xb = xb.reshape(N, ng, cg, 2, P, W).transpose(0, 1, 4, 2, 3, 5)
    xin = np.ascontiguousarray(xb).reshape(N, ng, P, cg * 2 * W)

    in_maps = []
    for i in range(NCORES):
        in_maps.append({"xin": xin[i], "th": th, "tw": tw})
    return in_maps


def _unpack_out(raw: np.ndarray, n_ch: int, cg: int = CG) -> np.ndarray:
    """[ng, P, cg*2*W] bf16 -> [n_ch, H, W] f32; h = s*128 + p."""
    ng = n_ch // cg
    r = raw.reshape(ng, P, cg, 2, W).transpose(0, 2, 3, 1, 4)
    return np.ascontiguousarray(r).reshape(n_ch, H, W).astype(np.float32)


def _run(x: np.ndarray, k2: np.ndarray, trace: bool = False):
    n_ch = C
    nc = _get_nc(n_ch)
    in_maps = _prep_inputs(x, k2, n_ch)
    r = run_bass_kernel_spmd(nc, in_maps, core_ids=list(range(NCORES)),
                             trace=trace)
    outs = [_unpack_out(r.results[i]["out"], n_ch) for i in range(NCORES)]
    return np.stack(outs, axis=0), r


def kernel(x: np.ndarray, kernel: np.ndarray) -> np.ndarray:
    out, _ = _run(x, kernel, trace=False)
    return out
